# revision 1
# baseline (speedup 1.0000x reference)
"""Trainium2 Bass kernel for a fused CustomLSTMCell.

Math (reference):
    combined = concat([input, hidden], axis=1)            # [B, D], D = 2048
    gates    = combined @ concat([W_i,W_f,W_o,W_g], 1) + b  # [B, 4H]
    i, f, o, g = split(gates, 4, axis=1)
    new_cell   = sigmoid(f) * cell_state + sigmoid(i) * tanh(g)
    new_hidden = sigmoid(o) * tanh(new_cell)

Strategy:
  - Data-parallel over batch: 8 cores x 1024 rows each. No collectives.
  - Host prepares A^T = combined.T (contraction dim D on partitions) in bf16
    and W in bf16; PSUM accumulates in fp32.
  - Per core GEMM: [1024, 2048] @ [2048, 4096] as 128x128x512 matmul tiles;
    the stationary operand is the A^T tile (reused across 4 gate tiles per
    K step), the moving operand is W.
  - Gate columns are processed in (i,f,o,g)-aligned groups of 512 so the
    elementwise LSTM math is local to one [128 x 512] tile set.
  - Bias add on DVE (PSUM + replicated-bias SBUF tile), sigmoid/tanh on ACT,
    cell/hidden updates on DVE, everything overlapped with the PE stream by
    the Tile scheduler.
"""

import sys

if "/opt/trn_rl_repo" not in sys.path:
    sys.path.insert(0, "/opt/trn_rl_repo")

import ml_dtypes
import numpy as np

import concourse.bass as bass
import concourse.mybir as mybir
import concourse.tile as tile
from concourse import bacc
from concourse.bass_utils import run_bass_kernel_spmd

N_CORES = 8
B = 8192
IN_SIZE = 1024
H = 1024
D = IN_SIZE + H          # 2048 contraction dim
G4 = 4 * H               # 4096 gate columns
BC = B // N_CORES        # 1024 batch rows per core
P = 128                  # partitions
KT = D // P              # 16 k-tiles
MT = BC // P             # 8 m-tiles (batch row tiles) per core
NG = 512                 # gate columns processed per group (per gate)
HGRPS = H // NG          # 2 column groups

_NC_CACHE = {}


def _build(iters: int = 1, loads_in_loop: bool = True, compute=True, ng: int = NG) -> bass.Bass:
    # compute: True = full body, False = no compute, "mm" = matmuls only
    hgrps = H // min(ng, 512)
    psum_bufs = 2 if ng <= 512 else 1
    nc = bacc.Bacc("TRN2", target_bir_lowering=False, debug=False)

    at = nc.dram_tensor("at", [D, BC], mybir.dt.bfloat16, kind="ExternalInput")
    w = nc.dram_tensor("w", [D, G4], mybir.dt.bfloat16, kind="ExternalInput")
    br = nc.dram_tensor("br", [P, G4], mybir.dt.float32, kind="ExternalInput")
    cs = nc.dram_tensor("cs", [BC, H], mybir.dt.float32, kind="ExternalInput")
    nh = nc.dram_tensor("nh", [BC, H], mybir.dt.float32, kind="ExternalOutput")
    ncl = nc.dram_tensor("ncl", [BC, H], mybir.dt.float32, kind="ExternalOutput")

    at_r = at.rearrange("(ko ki) b -> ki ko b", ki=P)   # [128, KT, BC]
    w_r = w.rearrange("(ko ki) n -> ki ko n", ki=P)     # [128, KT, G4]
    cs_r = cs.rearrange("(m p) h -> m p h", p=P)        # [MT, 128, H]
    nh_r = nh.rearrange("(m p) h -> m p h", p=P)
    ncl_r = ncl.rearrange("(m p) h -> m p h", p=P)

    AF = mybir.ActivationFunctionType

    from contextlib import nullcontext

    with tile.TileContext(nc) as tc:
        with (
            tc.tile_pool(name="resident", bufs=1) as rpool,
            tc.tile_pool(name="work", bufs=3) as wpool,
            tc.tile_pool(name="psum", bufs=psum_bufs, space="PSUM") as ppool,
        ):
            # benchmarking mode: repeat the whole body in-NEFF via a dynamic
            # loop (no instruction growth); iters=1 emits a straight-line body
            def loads():
                at_sb = rpool.tile([P, KT, BC], mybir.dt.bfloat16, tag="at", name="at_sb")
                w_sb = rpool.tile([P, KT, G4], mybir.dt.bfloat16, tag="w", name="w_sb")
                br_sb = rpool.tile([P, G4], mybir.dt.float32, tag="br", name="br_sb")
                nc.sync.dma_start(out=br_sb[:], in_=br[:])
                for k in range(KT):
                    nc.sync.dma_start(out=at_sb[:, k, :], in_=at_r[:, k, :])
                    nc.sync.dma_start(out=w_sb[:, k, :], in_=w_r[:, k, :])
                return at_sb, w_sb, br_sb

            if not loads_in_loop:
                at_sb, w_sb, br_sb = loads()
            with (tc.For_i(0, iters, 1) if iters > 1 else nullcontext()):
                if loads_in_loop:
                    at_sb, w_sb, br_sb = loads()
                for m in range(MT if compute else 0):
                    mm_groups = []
                    if ng <= 512:
                        for g in range(hgrps):
                            ps = [
                                ppool.tile([P, ng], mybir.dt.float32, tag=f"ps{q}", name=f"ps{q}")
                                for q in range(4)
                            ]
                            for k in range(KT):
                                lhs = at_sb[:, k, m * P : (m + 1) * P]
                                for q in range(4):
                                    nc.tensor.matmul(
                                        ps[q][:],
                                        lhsT=lhs,
                                        rhs=w_sb[:, k, q * H + g * ng : q * H + (g + 1) * ng],
                                        start=(k == 0),
                                        stop=(k == KT - 1),
                                    )
                            # psum slices for the elementwise stage, one per gate
                            mm_groups.append((g, [t[:] for t in ps]))
                    else:
                        # one 2-bank matmul per gate covers the full H columns
                        ps = [
                            ppool.tile([P, H], mybir.dt.float32, tag=f"ps{q}", name=f"ps{q}")
                            for q in range(4)
                        ]
                        for k in range(KT):
                            lhs = at_sb[:, k, m * P : (m + 1) * P]
                            for q in range(4):
                                nc.tensor.matmul(
                                    ps[q][:],
                                    lhsT=lhs,
                                    rhs=w_sb[:, k, q * H : (q + 1) * H],
                                    start=(k == 0),
                                    stop=(k == KT - 1),
                                )
                        for g in range(2):
                            mm_groups.append(
                                (g, [t[:, g * 512 : (g + 1) * 512] for t in ps])
                            )
                    if compute == "mm":
                        continue
                    for g, psl in mm_groups:
                        eg = 512 if ng > 512 else ng
                        cols = [q * H + g * eg for q in range(4)]
                        # bias add (DVE) + activation (ACT), PSUM -> SBUF
                        gt = [
                            wpool.tile([P, eg], mybir.dt.float32, tag=f"gt{q}", name=f"gt{q}")
                            for q in range(4)
                        ]
                        for q in range(4):
                            nc.vector.tensor_add(
                                out=gt[q][:],
                                in0=psl[q],
                                in1=br_sb[:, cols[q] : cols[q] + eg],
                            )
                        for q in range(3):
                            nc.scalar.activation(gt[q][:], gt[q][:], AF.Sigmoid)
                        nc.scalar.activation(gt[3][:], gt[3][:], AF.Tanh)

                        cl = wpool.tile([P, eg], mybir.dt.float32, tag="cl")
                        nc.sync.dma_start(
                            out=cl[:], in_=cs_r[m, :, g * eg : (g + 1) * eg]
                        )
                        si, sf, so, sg = gt
                        # new_cell = sigmoid(f)*c + sigmoid(i)*tanh(g)  -> sf
                        nc.vector.tensor_mul(out=sf[:], in0=sf[:], in1=cl[:])
                        nc.vector.tensor_mul(out=si[:], in0=si[:], in1=sg[:])
                        nc.vector.tensor_add(out=sf[:], in0=sf[:], in1=si[:])
                        # new_hidden = sigmoid(o)*tanh(new_cell)        -> so
                        nc.scalar.activation(sg[:], sf[:], AF.Tanh)
                        nc.vector.tensor_mul(out=so[:], in0=so[:], in1=sg[:])
                        nc.sync.dma_start(
                            out=ncl_r[m, :, g * eg : (g + 1) * eg], in_=sf[:]
                        )
                        nc.sync.dma_start(
                            out=nh_r[m, :, g * eg : (g + 1) * eg], in_=so[:]
                        )
    nc.finalize()
    return nc


def get_nc(iters: int = 1, loads_in_loop: bool = True, compute=True, ng: int = NG) -> bass.Bass:
    key = (iters, loads_in_loop, compute, ng)
    if key not in _NC_CACHE:
        _NC_CACHE[key] = _build(iters, loads_in_loop, compute, ng)
    return _NC_CACHE[key]


def make_in_maps(input, hidden, cell_state, W_i, b_i, W_f, b_f, W_o, b_o, W_g, b_g):
    comb = np.concatenate(
        [np.asarray(input, np.float32), np.asarray(hidden, np.float32)], axis=1
    )  # [B, D]
    W = np.concatenate(
        [np.asarray(W_i), np.asarray(W_f), np.asarray(W_o), np.asarray(W_g)], axis=1
    ).astype(np.float32)  # [D, 4H]
    b = np.concatenate(
        [np.asarray(b_i), np.asarray(b_f), np.asarray(b_o), np.asarray(b_g)]
    ).astype(np.float32)  # [4H]

    at_full = comb.T.astype(ml_dtypes.bfloat16)  # [D, B]
    w_bf = np.ascontiguousarray(W.astype(ml_dtypes.bfloat16))
    br = np.ascontiguousarray(np.broadcast_to(b, (P, G4)))
    cs = np.asarray(cell_state, np.float32)

    in_maps = []
    for c in range(N_CORES):
        sl = slice(c * BC, (c + 1) * BC)
        in_maps.append(
            {
                "at": np.ascontiguousarray(at_full[:, sl]),
                "w": w_bf,
                "br": br,
                "cs": np.ascontiguousarray(cs[sl]),
            }
        )
    return in_maps


def kernel(input, hidden, cell_state, W_i, b_i, W_f, b_f, W_o, b_o, W_g, b_g):
    in_maps = make_in_maps(
        input, hidden, cell_state, W_i, b_i, W_f, b_f, W_o, b_o, W_g, b_g
    )
    nc = get_nc(1)
    res = run_bass_kernel_spmd(nc, in_maps, core_ids=list(range(N_CORES)))
    new_hidden = np.concatenate(
        [res.results[c]["nh"] for c in range(N_CORES)], axis=0
    )
    new_cell = np.concatenate(
        [res.results[c]["ncl"] for c in range(N_CORES)], axis=0
    )
    return new_hidden, new_cell



# revision 4
# speedup vs baseline: 1.0607x; 1.0607x over previous
"""Trainium2 Bass kernel for a fused CustomLSTMCell.

Math (reference):
    combined = concat([input, hidden], axis=1)            # [B, D], D = 2048
    gates    = combined @ concat([W_i,W_f,W_o,W_g], 1) + b  # [B, 4H]
    i, f, o, g = split(gates, 4, axis=1)
    new_cell   = sigmoid(f) * cell_state + sigmoid(i) * tanh(g)
    new_hidden = sigmoid(o) * tanh(new_cell)

Strategy:
  - Data-parallel over batch: 8 cores x 1024 rows each. No collectives.
  - Host prepares A^T = combined.T (contraction dim D on partitions) in bf16
    and W in bf16 packed as two half-of-H column blocks; PSUM accumulates fp32.
  - Loop nest is g-outer (gate-column half) so only half of W (8 MB) must be
    resident at a time; the two halves alternate SBUF buffers, so the 16 MB
    W stream overlaps compute both within an iteration and across the For_i
    back-edge (the m-outer variant starved the PE ~30 us/iter waiting on W).
  - Per (g, m): 4 PSUM banks accumulate the i/f/o/g gate tiles over 16
    k-steps (stationary = A^T tile, reused across the 4 gates; moving = W).
  - Bias add on DVE (PSUM + replicated bf16 bias), sigmoid/tanh on ACT,
    cell/hidden updates on DVE, all overlapped with the PE stream.
"""

import sys

if "/opt/trn_rl_repo" not in sys.path:
    sys.path.insert(0, "/opt/trn_rl_repo")

import ml_dtypes
import numpy as np

import concourse.bass as bass
import concourse.mybir as mybir
import concourse.tile as tile
from concourse import bacc
from concourse.bass_utils import run_bass_kernel_spmd

N_CORES = 8
B = 8192
IN_SIZE = 1024
H = 1024
D = IN_SIZE + H          # 2048 contraction dim
G4 = 4 * H               # 4096 gate columns
BC = B // N_CORES        # 1024 batch rows per core
P = 128                  # partitions
KT = D // P              # 16 k-tiles
MT = BC // P             # 8 m-tiles (batch row tiles) per core
NG = 512                 # gate columns per (g, q) tile
WCOL = 4 * NG            # 2048 W columns per g-block

_NC_CACHE = {}


def _build(iters: int = 1) -> bass.Bass:
    nc = bacc.Bacc("TRN2", target_bir_lowering=False, debug=False)

    at = nc.dram_tensor("at", [D, BC], mybir.dt.bfloat16, kind="ExternalInput")
    w = nc.dram_tensor("w", [2, P, KT * WCOL], mybir.dt.bfloat16, kind="ExternalInput")
    br = nc.dram_tensor("br", [P, G4], mybir.dt.bfloat16, kind="ExternalInput")
    cs = nc.dram_tensor("cs", [BC, H], mybir.dt.float32, kind="ExternalInput")
    nh = nc.dram_tensor("nh", [BC, H], mybir.dt.float32, kind="ExternalOutput")
    ncl = nc.dram_tensor("ncl", [BC, H], mybir.dt.float32, kind="ExternalOutput")

    at_r = at.rearrange("(ko ki) b -> ki ko b", ki=P)   # [128, KT, BC]
    cs_r = cs.rearrange("(m p) h -> m p h", p=P)        # [MT, 128, H]
    nh_r = nh.rearrange("(m p) h -> m p h", p=P)
    ncl_r = ncl.rearrange("(m p) h -> m p h", p=P)

    AF = mybir.ActivationFunctionType

    from contextlib import nullcontext

    with tile.TileContext(nc) as tc:
        with (
            tc.tile_pool(name="resident", bufs=1) as rpool,
            tc.tile_pool(name="work", bufs=2) as wpool,
            tc.tile_pool(name="psum", bufs=2, space="PSUM") as ppool,
        ):
            with (tc.For_i(0, iters, 1) if iters > 1 else nullcontext()):
                br_sb = rpool.tile([P, G4], mybir.dt.bfloat16, tag="br")
                nc.sync.dma_start(out=br_sb[:], in_=br[:])
                at_sb = rpool.tile([P, KT, BC], mybir.dt.bfloat16, tag="at")
                for g in range(2):
                    # stream this g-block of W (8 MB) into its own buffer,
                    # interleaved per-k with the at loads (g=0 only) so the
                    # first m-tile's k-steps never starve
                    wg_sb = rpool.tile(
                        [P, KT, WCOL], mybir.dt.bfloat16, tag=f"wg{g}"
                    )
                    for k in range(KT):
                        nc.sync.dma_start(
                            out=wg_sb[:, k, :],
                            in_=w[g, :, k * WCOL : (k + 1) * WCOL],
                        )
                        if g == 0:
                            nc.sync.dma_start(out=at_sb[:, k, :], in_=at_r[:, k, :])
                    for m in range(MT):
                        ps = [
                            ppool.tile([P, NG], mybir.dt.float32, tag=f"ps{q}", name=f"ps{q}")
                            for q in range(4)
                        ]
                        for k in range(KT):
                            lhs = at_sb[:, k, m * P : (m + 1) * P]
                            for q in range(4):
                                nc.tensor.matmul(
                                    ps[q][:],
                                    lhsT=lhs,
                                    rhs=wg_sb[:, k, q * NG : (q + 1) * NG],
                                    start=(k == 0),
                                    stop=(k == KT - 1),
                                )
                        # bias add (DVE) + activation (ACT), PSUM -> SBUF
                        cols = [q * H + g * NG for q in range(4)]
                        gt = [
                            wpool.tile([P, NG], mybir.dt.float32, tag=f"gt{q}", name=f"gt{q}")
                            for q in range(4)
                        ]
                        for q in range(4):
                            nc.vector.tensor_add(
                                out=gt[q][:],
                                in0=ps[q][:],
                                in1=br_sb[:, cols[q] : cols[q] + NG],
                            )
                        for q in range(3):
                            nc.scalar.activation(gt[q][:], gt[q][:], AF.Sigmoid)
                        nc.scalar.activation(gt[3][:], gt[3][:], AF.Tanh)

                        cl = wpool.tile([P, NG], mybir.dt.float32, tag="cl")
                        nc.sync.dma_start(
                            out=cl[:], in_=cs_r[m, :, g * NG : (g + 1) * NG]
                        )
                        si, sf, so, sg = gt
                        # new_cell = sigmoid(f)*c + sigmoid(i)*tanh(g)  -> sf
                        nc.vector.tensor_mul(out=sf[:], in0=sf[:], in1=cl[:])
                        nc.vector.tensor_mul(out=si[:], in0=si[:], in1=sg[:])
                        nc.vector.tensor_add(out=sf[:], in0=sf[:], in1=si[:])
                        # new_hidden = sigmoid(o)*tanh(new_cell)        -> so
                        nc.scalar.activation(sg[:], sf[:], AF.Tanh)
                        nc.vector.tensor_mul(out=so[:], in0=so[:], in1=sg[:])
                        nc.sync.dma_start(
                            out=ncl_r[m, :, g * NG : (g + 1) * NG], in_=sf[:]
                        )
                        nc.sync.dma_start(
                            out=nh_r[m, :, g * NG : (g + 1) * NG], in_=so[:]
                        )
    nc.finalize()
    return nc


def get_nc(iters: int = 1) -> bass.Bass:
    if iters not in _NC_CACHE:
        _NC_CACHE[iters] = _build(iters)
    return _NC_CACHE[iters]


def make_in_maps(input, hidden, cell_state, W_i, b_i, W_f, b_f, W_o, b_o, W_g, b_g):
    comb = np.concatenate(
        [np.asarray(input, np.float32), np.asarray(hidden, np.float32)], axis=1
    )  # [B, D]
    W4 = np.concatenate(
        [np.asarray(W_i), np.asarray(W_f), np.asarray(W_o), np.asarray(W_g)], axis=1
    ).astype(np.float32)  # [D, 4H]
    b = np.concatenate(
        [np.asarray(b_i), np.asarray(b_f), np.asarray(b_o), np.asarray(b_g)]
    ).astype(np.float32)  # [4H]

    at_full = comb.T.astype(ml_dtypes.bfloat16)  # [D, B]
    # pack W into two g-blocks: w[g] = [128 ki, KT*WCOL] with row ki holding
    # the (ko, q, col) line so each per-k DMA slice is contiguous
    wg = np.empty((2, P, KT * WCOL), dtype=ml_dtypes.bfloat16)
    for g in range(2):
        blk = np.concatenate(
            [W4[:, q * H + g * NG : q * H + (g + 1) * NG] for q in range(4)], axis=1
        )  # [D, WCOL]
        wg[g] = (
            blk.astype(ml_dtypes.bfloat16)
            .reshape(KT, P, WCOL)
            .transpose(1, 0, 2)
            .reshape(P, KT * WCOL)
        )
    br = np.ascontiguousarray(
        np.broadcast_to(b.astype(ml_dtypes.bfloat16), (P, G4))
    )
    cs = np.asarray(cell_state, np.float32)

    in_maps = []
    for c in range(N_CORES):
        sl = slice(c * BC, (c + 1) * BC)
        in_maps.append(
            {
                "at": np.ascontiguousarray(at_full[:, sl]),
                "w": wg,
                "br": br,
                "cs": np.ascontiguousarray(cs[sl]),
            }
        )
    return in_maps


def kernel(input, hidden, cell_state, W_i, b_i, W_f, b_f, W_o, b_o, W_g, b_g):
    in_maps = make_in_maps(
        input, hidden, cell_state, W_i, b_i, W_f, b_f, W_o, b_o, W_g, b_g
    )
    nc = get_nc(1)
    res = run_bass_kernel_spmd(nc, in_maps, core_ids=list(range(N_CORES)))
    new_hidden = np.concatenate(
        [res.results[c]["nh"] for c in range(N_CORES)], axis=0
    )
    new_cell = np.concatenate(
        [res.results[c]["ncl"] for c in range(N_CORES)], axis=0
    )
    return new_hidden, new_cell


# revision 6
# speedup vs baseline: 1.0735x; 1.0121x over previous
"""Trainium2 Bass kernel for a fused CustomLSTMCell.

Math (reference):
    combined = concat([input, hidden], axis=1)            # [B, D], D = 2048
    gates    = combined @ concat([W_i,W_f,W_o,W_g], 1) + b  # [B, 4H]
    i, f, o, g = split(gates, 4, axis=1)
    new_cell   = sigmoid(f) * cell_state + sigmoid(i) * tanh(g)
    new_hidden = sigmoid(o) * tanh(new_cell)

Strategy:
  - Data-parallel over batch: 8 cores x 1024 rows each. No collectives.
  - Transposed W-stationary layout: gate columns (H) live on PSUM partitions,
    batch on the free dim. gates^T = W^T @ combined^T per 128-column h-block.
    Host prepares combined^T (bf16), W packed per h-block, cell_state^T.
  - The per-gate bias is then per-partition, so it folds into the ACT
    activation (out = sigmoid/tanh(psum + bias)) -- no DVE bias adds.
  - The cell/hidden elementwise chain runs on DVE in bf16 (2x rate). fp32
    drain math measured ~250us of DVE occupancy and periodically stalled the
    PE by delaying PSUM recycling; bf16 + ACT-bias cuts it to ~30us.
  - W streams as 8 x 2MB h-blocks through a 4-deep buffer ring, per-k DMA
    slices interleaved with the combined^T loads, so the PE never starves
    and the For_i back-edge overlaps.
"""

import sys

if "/opt/trn_rl_repo" not in sys.path:
    sys.path.insert(0, "/opt/trn_rl_repo")

import ml_dtypes
import numpy as np

import concourse.bass as bass
import concourse.mybir as mybir
import concourse.tile as tile
from concourse import bacc
from concourse.bass_utils import run_bass_kernel_spmd

N_CORES = 8
B = 8192
IN_SIZE = 1024
H = 1024
D = IN_SIZE + H          # 2048 contraction dim
G4 = 4 * H               # 4096 gate columns
BC = B // N_CORES        # 1024 batch rows per core
P = 128                  # partitions
KT = D // P              # 16 k-tiles
JT = H // P              # 8 h-blocks
NB = 512                 # batch columns per matmul (moving free dim)
BBLK = BC // NB          # 2 batch blocks

_NC_CACHE = {}


def _build(iters: int = 1) -> bass.Bass:
    nc = bacc.Bacc("TRN2", target_bir_lowering=False, debug=False)

    at = nc.dram_tensor("at", [D, BC], mybir.dt.bfloat16, kind="ExternalInput")
    wj = nc.dram_tensor("wj", [JT, P, KT * 512], mybir.dt.bfloat16, kind="ExternalInput")
    bv = nc.dram_tensor("bv", [P, 4 * JT], mybir.dt.float32, kind="ExternalInput")
    cst = nc.dram_tensor("cst", [JT, P, BC], mybir.dt.bfloat16, kind="ExternalInput")
    nht = nc.dram_tensor("nht", [JT, P, BC], mybir.dt.bfloat16, kind="ExternalOutput")
    nclt = nc.dram_tensor("nclt", [JT, P, BC], mybir.dt.bfloat16, kind="ExternalOutput")

    at_r = at.rearrange("(ko ki) b -> ki ko b", ki=P)   # [128, KT, BC]

    AF = mybir.ActivationFunctionType

    from contextlib import nullcontext

    with tile.TileContext(nc) as tc:
        with (
            tc.tile_pool(name="resident", bufs=1) as rpool,
            tc.tile_pool(name="wstream", bufs=4) as wspool,
            tc.tile_pool(name="work", bufs=2) as wpool,
            tc.tile_pool(name="psum", bufs=2, space="PSUM") as ppool,
        ):
            with (tc.For_i(0, iters, 1) if iters > 1 else nullcontext()):
                bv_sb = rpool.tile([P, 4 * JT], mybir.dt.float32, tag="bv")
                nc.sync.dma_start(out=bv_sb[:], in_=bv[:])
                at_sb = rpool.tile([P, KT, BC], mybir.dt.bfloat16, tag="at")
                for j in range(JT):
                    # stream this h-block of W (2 MB), interleaved per-k with
                    # the combined^T loads on the first block
                    wj_sb = wspool.tile([P, KT, 512], mybir.dt.bfloat16, tag="wj", name="wj_sb")
                    for k in range(KT):
                        nc.sync.dma_start(
                            out=wj_sb[:, k, :], in_=wj[j, :, k * 512 : (k + 1) * 512]
                        )
                        if j == 0:
                            nc.sync.dma_start(out=at_sb[:, k, :], in_=at_r[:, k, :])
                    for b in range(BBLK):
                        # prefetch the cell-state block a full group early
                        cl = wpool.tile([P, NB], mybir.dt.bfloat16, tag="cl")
                        nc.sync.dma_start(
                            out=cl[:], in_=cst[j, :, b * NB : (b + 1) * NB]
                        )
                        ps = [
                            ppool.tile([P, NB], mybir.dt.float32, tag=f"ps{q}", name=f"ps{q}")
                            for q in range(4)
                        ]
                        for k in range(KT):
                            mov = at_sb[:, k, b * NB : (b + 1) * NB]
                            for q in range(4):
                                nc.tensor.matmul(
                                    ps[q][:],
                                    lhsT=wj_sb[:, k, q * P : (q + 1) * P],
                                    rhs=mov,
                                    start=(k == 0),
                                    stop=(k == KT - 1),
                                )
                        # gate activations with fused per-partition bias,
                        # PSUM -> SBUF bf16
                        gt = [
                            wpool.tile([P, NB], mybir.dt.bfloat16, tag=f"gt{q}", name=f"gt{q}")
                            for q in range(4)
                        ]
                        for q in range(4):
                            nc.scalar.activation(
                                gt[q][:],
                                ps[q][:],
                                AF.Tanh if q == 3 else AF.Sigmoid,
                                bias=bv_sb[:, q * JT + j : q * JT + j + 1],
                            )
                        si, sf, so, sg = gt
                        # new_cell = sigmoid(f)*c + sigmoid(i)*tanh(g)  -> sf
                        nc.vector.tensor_mul(out=sf[:], in0=sf[:], in1=cl[:])
                        nc.vector.tensor_mul(out=si[:], in0=si[:], in1=sg[:])
                        nc.vector.tensor_add(out=sf[:], in0=sf[:], in1=si[:])
                        # new_hidden = sigmoid(o)*tanh(new_cell)        -> so
                        nc.scalar.activation(sg[:], sf[:], AF.Tanh)
                        nc.vector.tensor_mul(out=so[:], in0=so[:], in1=sg[:])
                        nc.sync.dma_start(
                            out=nclt[j, :, b * NB : (b + 1) * NB], in_=sf[:]
                        )
                        nc.sync.dma_start(
                            out=nht[j, :, b * NB : (b + 1) * NB], in_=so[:]
                        )
    nc.finalize()
    return nc


def get_nc(iters: int = 1) -> bass.Bass:
    if iters not in _NC_CACHE:
        _NC_CACHE[iters] = _build(iters)
    return _NC_CACHE[iters]


def make_in_maps(input, hidden, cell_state, W_i, b_i, W_f, b_f, W_o, b_o, W_g, b_g):
    comb = np.concatenate(
        [np.asarray(input, np.float32), np.asarray(hidden, np.float32)], axis=1
    )  # [B, D]
    W4 = np.concatenate(
        [np.asarray(W_i), np.asarray(W_f), np.asarray(W_o), np.asarray(W_g)], axis=1
    ).astype(np.float32)  # [D, 4H]
    b = np.concatenate(
        [np.asarray(b_i), np.asarray(b_f), np.asarray(b_o), np.asarray(b_g)]
    ).astype(np.float32)  # [4H]

    at_full = comb.T.astype(ml_dtypes.bfloat16)  # [D, B]
    # W packed per h-block j: [128 ki, KT, (q, col)] so per-k DMA slices are
    # contiguous and the stationary operand for (k, q) is wj[:, k, q*128:...]
    wj = np.empty((JT, P, KT * 512), dtype=ml_dtypes.bfloat16)
    for j in range(JT):
        blk = np.concatenate(
            [W4[:, q * H + j * P : q * H + (j + 1) * P] for q in range(4)], axis=1
        )  # [D, 512], col = q*128 + c
        wj[j] = (
            blk.astype(ml_dtypes.bfloat16)
            .reshape(KT, P, 512)
            .transpose(1, 0, 2)
            .reshape(P, KT * 512)
        )
    bv = np.ascontiguousarray(
        b.reshape(4, JT, P).transpose(2, 0, 1).reshape(P, 4 * JT)
    )
    cs = np.asarray(cell_state, np.float32)

    in_maps = []
    for c in range(N_CORES):
        sl = slice(c * BC, (c + 1) * BC)
        cst = np.ascontiguousarray(
            cs[sl].T.reshape(JT, P, BC).astype(ml_dtypes.bfloat16)
        )
        in_maps.append(
            {
                "at": np.ascontiguousarray(at_full[:, sl]),
                "wj": wj,
                "bv": bv,
                "cst": cst,
            }
        )
    return in_maps


def kernel(input, hidden, cell_state, W_i, b_i, W_f, b_f, W_o, b_o, W_g, b_g):
    in_maps = make_in_maps(
        input, hidden, cell_state, W_i, b_i, W_f, b_f, W_o, b_o, W_g, b_g
    )
    nc = get_nc(1)
    res = run_bass_kernel_spmd(nc, in_maps, core_ids=list(range(N_CORES)))
    new_hidden = np.concatenate(
        [
            res.results[c]["nht"].astype(np.float32).reshape(H, BC).T
            for c in range(N_CORES)
        ],
        axis=0,
    )
    new_cell = np.concatenate(
        [
            res.results[c]["nclt"].astype(np.float32).reshape(H, BC).T
            for c in range(N_CORES)
        ],
        axis=0,
    )
    return new_hidden, new_cell
